# revision 1
# baseline (speedup 1.0000x reference)
"""Multi-head attention (B=2, S=2048, D=1024, H=16, d_k=64) on 8 NeuronCores.

Sharding: 8 cores = 2 batches x 4 head-groups (4 heads each).
Core c handles batch b = c//4 and heads 4*(c%4) .. 4*(c%4)+4 (feature
slice of width F=256). Each core computes its partial output-projection
contribution [S, D]; the host sums the 4 head-group partials per batch
and adds b4 (the "all-reduce" of the row-sharded W4 projection).

Device dataflow works in a "transposed world" so every matmul operand
is in its natural PE layout (contraction on partitions), with zero
on-device transposes:
  qT = W1g @ x_q.T  [F, S]   (lhsT = W1g.T host-prepped, rhs = x_q.T host-prepped)
  kT = W2g @ x_k.T  [F, S]
  v  = x_v @ W3g.T  [S, F]   (lhsT = x_v.T, rhs = W3g.T; bias via K=1 ones matmul)
  scoresT_h = kT_h.T @ qT_h        [S_keys, S_q]   (K = d_k = 64; 2 heads packed
                                                    in PE row groups 0:64 / 64:128)
  attnT = exp(scoresT / 8)          ACT, PSUM->SBUF bf16, no max subtraction
                                    (scores ~ N(0,1); max|score/8| ~ 10 -> safe in f32)
  pv = [v_h | ones].T @ attnT      [65, S_q]; row 64 = softmax denominator
  outT_h = pv[0:64] * (1/pv[64])   per-query normalization (flash-style, post-PV)
  partial = outT.T @ W4g.T         [S, D]  (lhsT = outT, rhs = W4g.T host-prepped)

All matmuls bf16 with f32 PSUM accumulation (validated 0.6% L2 rel err).
"""

import numpy as np
import ml_dtypes

import concourse.bass as bass
import concourse.mybir as mybir
import concourse.tile as tile
from concourse import bacc
from concourse.bass_utils import run_bass_kernel_spmd

BF16 = ml_dtypes.bfloat16
F32 = mybir.dt.float32
BF = mybir.dt.bfloat16

B, S, D = 2, 2048, 1024
H_CORE = 4          # heads per core
DK = 64             # head dim
F = H_CORE * DK     # features per core = 256
P = 128             # partitions
KB = D // P         # k blocks in D contraction = 8
SM = S // P         # seq tiles of 128 = 16
QC = S // 512       # query chunks of 512 = 4
N_CORES = 8


def _build_kernel():
    nc = bacc.Bacc(
        "TRN2",
        target_bir_lowering=False,
        debug=False,
        enable_asserts=False,
        num_devices=N_CORES,
    )

    xq = nc.dram_tensor("xq_t", [D, S], BF, kind="ExternalInput").ap()
    xk = nc.dram_tensor("xk_t", [D, S], BF, kind="ExternalInput").ap()
    xv = nc.dram_tensor("xv_t", [D, S], BF, kind="ExternalInput").ap()
    w1 = nc.dram_tensor("w1t", [D, F], BF, kind="ExternalInput").ap()
    w2 = nc.dram_tensor("w2t", [D, F], BF, kind="ExternalInput").ap()
    w3 = nc.dram_tensor("w3t", [D, F], BF, kind="ExternalInput").ap()
    w4 = nc.dram_tensor("w4t", [F, D], BF, kind="ExternalInput").ap()
    b1 = nc.dram_tensor("b1c", [P, F // P], F32, kind="ExternalInput").ap()
    b2 = nc.dram_tensor("b2c", [P, F // P], F32, kind="ExternalInput").ap()
    b3 = nc.dram_tensor("b3r", [1, F], BF, kind="ExternalInput").ap()
    out = nc.dram_tensor("out", [S, D], F32, kind="ExternalOutput").ap()

    with tile.TileContext(nc) as tc:
        _body(tc, xq, xk, xv, w1, w2, w3, w4, b1, b2, b3, out)

    nc.compile()
    return nc


def _body(tc, xq, xk, xv, w1, w2, w3, w4, b1, b2, b3, out):
    nc = tc.nc
    MF = F // P  # m tiles for the F=256 feature dim = 2

    with (
        tc.tile_pool(name="wpool", bufs=1) as wpool,
        tc.tile_pool(name="xt", bufs=KB) as xt_pool,
        tc.tile_pool(name="persist", bufs=1) as persist,
        tc.tile_pool(name="attn", bufs=1) as attn_pool,
        tc.tile_pool(name="small", bufs=4) as small,
        tc.tile_pool(name="stage", bufs=4) as stage,
        tc.tile_pool(name="psum", bufs=1, space="PSUM") as psum,
    ):
        # ---- weights / constants to SBUF ----
        w1_sb = [wpool.tile([P, F], BF, name=f"w1_{k}", tag=f"w1_{k}") for k in range(KB)]
        w2_sb = [wpool.tile([P, F], BF, name=f"w2_{k}", tag=f"w2_{k}") for k in range(KB)]
        w3_sb = [wpool.tile([P, F], BF, name=f"w3_{k}", tag=f"w3_{k}") for k in range(KB)]
        w4_sb = [wpool.tile([P, D], BF, name=f"w4_{k}", tag=f"w4_{k}") for k in range(MF)]
        for k in range(KB):
            nc.sync.dma_start(w1_sb[k][:], w1[k * P:(k + 1) * P, :])
            nc.sync.dma_start(w2_sb[k][:], w2[k * P:(k + 1) * P, :])
            nc.sync.dma_start(w3_sb[k][:], w3[k * P:(k + 1) * P, :])
        for k in range(MF):
            nc.sync.dma_start(w4_sb[k][:], w4[k * P:(k + 1) * P, :])
        b1_sb = wpool.tile([P, MF], F32, name="b1_sb", tag="b1_sb")
        b2_sb = wpool.tile([P, MF], F32, name="b2_sb", tag="b2_sb")
        b3_sb = wpool.tile([1, F], BF, name="b3_sb", tag="b3_sb")
        nc.sync.dma_start(b1_sb[:], b1[:])
        nc.sync.dma_start(b2_sb[:], b2[:])
        nc.sync.dma_start(b3_sb[:], b3[:])
        ones_row = wpool.tile([1, P], BF, name="ones_row", tag="ones_row")
        nc.vector.memset(ones_row[:], 1.0)

        # persistent activations
        qT = [persist.tile([P, S], BF, name=f"qT_{m}", tag=f"qT_{m}") for m in range(MF)]
        kT = [persist.tile([P, S], BF, name=f"kT_{m}", tag=f"kT_{m}") for m in range(MF)]
        # v with interleaved ones columns: per head h, cols 65h..65h+63 = v_h,
        # col 65h+64 = 1.0 (softmax denominator trick)
        VW = H_CORE * (DK + 1)  # 260
        v_sb = [persist.tile([P, VW], BF, name=f"v_{s}", tag=f"v_{s}") for s in range(SM)]
        for s in range(SM):
            for h in range(H_CORE):
                nc.vector.memset(v_sb[s][:, h * (DK + 1) + DK: h * (DK + 1) + DK + 1], 1.0)
        outT = [persist.tile([P, S], BF, name=f"outT_{m}", tag=f"outT_{m}") for m in range(MF)]

        # ---- q / k projections: qT[m][:, qc] = sum_k W1t[k][:,m].T @ xq[k][:,qc] ----
        for name, x_dram, w_sb, b_sb, dst in (
            ("q", xq, w1_sb, b1_sb, qT),
            ("k", xk, w2_sb, b2_sb, kT),
        ):
            x_sb = []
            for k in range(KB):
                t = xt_pool.tile([P, S], BF, name=f"x{name}_{k}", tag="xt")
                nc.sync.dma_start(t[:], x_dram[k * P:(k + 1) * P, :])
                x_sb.append(t)
            for m in range(MF):
                for qc in range(QC):
                    ps = psum.tile([P, 512], F32, name=f"pp_{name}_{m}_{qc}", tag="pp", bufs=2)
                    for k in range(KB):
                        nc.tensor.matmul(
                            ps[:],
                            w_sb[k][:, m * P:(m + 1) * P],
                            x_sb[k][:, qc * 512:(qc + 1) * 512],
                            start=(k == 0),
                            stop=(k == KB - 1),
                        )
                    nc.vector.tensor_scalar_add(
                        dst[m][:, qc * 512:(qc + 1) * 512], ps[:], b_sb[:, m:m + 1]
                    )

        # ---- v projection (natural layout): v[s] = xv[:, s].T @ W3t + b3 ----
        x_sb = []
        for k in range(KB):
            t = xt_pool.tile([P, S], BF, name=f"xv_{k}", tag="xt")
            nc.sync.dma_start(t[:], xv[k * P:(k + 1) * P, :])
            x_sb.append(t)
        for s in range(SM):
            ps = psum.tile([P, F], F32, name=f"pv_{s}", tag="pp", bufs=2)
            for k in range(KB):
                nc.tensor.matmul(
                    ps[:],
                    x_sb[k][:, s * P:(s + 1) * P],
                    w3_sb[k][:],
                    start=(k == 0),
                    stop=False,
                )
            # bias: += ones.T @ b3  (K=1)
            nc.tensor.matmul(ps[:], ones_row[:], b3_sb[:], start=False, stop=True)
            for h in range(H_CORE):
                nc.vector.tensor_copy(
                    v_sb[s][:, h * (DK + 1): h * (DK + 1) + DK],
                    ps[:, h * DK:(h + 1) * DK],
                )

        # ---- attention, per head-pair hp (heads 2hp, 2hp+1 live in qT/kT tile hp) ----
        for hp in range(MF):
            for qc in range(QC):
                qsl = slice(qc * 512, (qc + 1) * 512)
                attn_t = [[None] * SM for _ in range(2)]
                for kt in range(SM):
                    for h2 in range(2):
                        rsl = slice(h2 * DK, (h2 + 1) * DK)
                        ps = psum.tile([P, 512], F32, name=f"sc_{hp}_{qc}_{kt}_{h2}",
                                       tag="sc", bufs=4)
                        nc.tensor.matmul(
                            ps[:],
                            kT[hp][rsl, kt * P:(kt + 1) * P],
                            qT[hp][rsl, qsl],
                            start=True,
                            stop=True,
                        )
                        at = attn_pool.tile([P, 512], BF, name=f"at_{hp}_{qc}_{kt}_{h2}",
                                            tag="attnT", bufs=64)
                        nc.scalar.activation(
                            at[:], ps[:], mybir.ActivationFunctionType.Exp,
                            scale=1.0 / np.sqrt(DK),
                        )
                        attn_t[h2][kt] = at
                for h2 in range(2):
                    h = hp * 2 + h2
                    vsl = slice(h * (DK + 1), h * (DK + 1) + DK + 1)
                    pv = psum.tile([P, 512], F32, name=f"pvps_{hp}_{qc}_{h2}",
                                   tag="pv", bufs=2)
                    for kt in range(SM):
                        nc.tensor.matmul(
                            pv[0:DK + 1, :],
                            v_sb[kt][:, vsl],
                            attn_t[h2][kt][:],
                            start=(kt == 0),
                            stop=(kt == SM - 1),
                        )
                    rc = small.tile([1, 512], F32, name=f"rc_{hp}_{qc}_{h2}", tag="rc")
                    nc.vector.reciprocal(rc[:], pv[DK:DK + 1, :])
                    bc = small.tile([DK, 512], F32, name=f"bc_{hp}_{qc}_{h2}", tag="bc")
                    nc.gpsimd.partition_broadcast(bc[:], rc[:])
                    nc.vector.tensor_mul(
                        outT[hp][h2 * DK:(h2 + 1) * DK, qsl], pv[0:DK, :], bc[:]
                    )

        # ---- output projection partial: out[qt, oc] = sum_m outT[m][:,qt].T @ w4[m][:,oc] ----
        for qt in range(SM):
            for oc in range(D // 512):
                ps = psum.tile([P, 512], F32, name=f"po_{qt}_{oc}", tag="pp", bufs=2)
                for m in range(MF):
                    nc.tensor.matmul(
                        ps[:],
                        outT[m][:, qt * P:(qt + 1) * P],
                        w4_sb[m][:, oc * 512:(oc + 1) * 512],
                        start=(m == 0),
                        stop=(m == MF - 1),
                    )
                ob = stage.tile([P, 512], F32, name=f"ob_{qt}_{oc}", tag="ob")
                nc.vector.tensor_copy(ob[:], ps[:])
                nc.sync.dma_start(out[qt * P:(qt + 1) * P, oc * 512:(oc + 1) * 512], ob[:])


_NC_CACHE = None


def _get_nc():
    global _NC_CACHE
    if _NC_CACHE is None:
        _NC_CACHE = _build_kernel()
    return _NC_CACHE


def _make_in_maps(query, key, value, W1, b1, W2, b2, W3, b3, W4, b4):
    in_maps = []
    for c in range(N_CORES):
        b, g = divmod(c, 4)
        gs = slice(g * F, (g + 1) * F)
        in_maps.append({
            "xq_t": np.ascontiguousarray(query[b].T).astype(BF16),
            "xk_t": np.ascontiguousarray(key[b].T).astype(BF16),
            "xv_t": np.ascontiguousarray(value[b].T).astype(BF16),
            "w1t": np.ascontiguousarray(W1[gs, :].T).astype(BF16),
            "w2t": np.ascontiguousarray(W2[gs, :].T).astype(BF16),
            "w3t": np.ascontiguousarray(W3[gs, :].T).astype(BF16),
            "w4t": np.ascontiguousarray(W4[:, gs].T).astype(BF16),
            "b1c": np.ascontiguousarray(b1[gs].reshape(F // P, P).T).astype(np.float32),
            "b2c": np.ascontiguousarray(b2[gs].reshape(F // P, P).T).astype(np.float32),
            "b3r": b3[gs].reshape(1, F).astype(BF16),
        })
    return in_maps


def kernel(query, key, value, W1, b1, W2, b2, W3, b3, W4, b4, _trace=False, _tmpdir=None):
    args = [np.asarray(a) for a in (query, key, value, W1, b1, W2, b2, W3, b3, W4, b4)]
    nc = _get_nc()
    in_maps = _make_in_maps(*args)
    res = run_bass_kernel_spmd(
        nc, in_maps, core_ids=list(range(N_CORES)),
        trace=_trace, tmpdir=_tmpdir,
    )
    b4_f = args[10].astype(np.float32)
    full = np.zeros((B, S, D), np.float32)
    for c in range(N_CORES):
        full[c // 4] += res.results[c]["out"]
    full += b4_f[None, None, :]
    kernel.last_results = res
    return full
